# revision 12
# baseline (speedup 1.0000x reference)
"""AdversarialBlockShift on 8 TRN2 NeuronCores.

The module's learnable `param` is a one-hot shift selector (a delta at
index `max_left_shift` at init).  After F.pad + flip, the depthwise
conv kernel `pk` is a delta at position k0, so the conv over the user
span is a pure shift by d = k0 - Kp//2 (zeros shifted in at the edge),
and the id path is the matching block move of the adversarial ids.

Everything therefore reduces to one (B*S, D) gather out of the
embedding table with host-computable row indices:

  host:   O(S) int index bookkeeping (shift map, run structure, ids)
  device: the 32 MiB of real memory traffic — gather 8192 rows x 2 KiB
          from the (32000, 512) fp32 table and stream them back out —
          data-parallel over 8 cores, 1024 rows (2+2 MiB) per core.

Per core: SWDGE indirect-DMA gather (128 rows per descriptor batch,
one 2 KiB descriptor per row) into SBUF, HWDGE store to the output
shard, pipelined in chunks so gather-in and store-out overlap.
"""

import numpy as np

import concourse.bass as bass
import concourse.bacc as bacc
from concourse import mybir
from concourse.bass_utils import run_bass_kernel_spmd

# -------- problem constants (hardcoded per contest contract) --------
B, S, D, V = 2, 4096, 512, 32000
N_CORES = 8
ROWS = B * S                      # 8192 flattened output rows
RPC = ROWS // N_CORES             # 1024 rows per core
JPC = RPC // 128                  # 8 row-blocks of 128 per core
# HW indirect DMA gathers ONE row per partition per instruction, so a
# core's 1024 rows take JPC=8 gather instructions of [128, D] each.
# Row mapping (j-major): local row r = j*128 + p  <=>  idx_t[p, j].

# set by test.py for profiling; harness never sets it
TRACE = {"enabled": False, "kwargs": {}}
LAST_RESULTS = {}

_prog_cache = {}


def _build_program(table_rows):
    """Fast path: custom SWDGE dma_gather (Q7 'mlp' library kernel).

    One descriptor per gathered row, but descriptor generation runs at the
    custom-kernel rate (~0.34 ns/desc) instead of the generic indirect-DMACopy
    rate (~10 ns/desc).  Indices are int16, so this path requires
    table_rows <= 32767 (true for the real input; the augmented-table variants
    fall back to the generic indirect path below).

    dma_gather semantics (non-transpose):
      out[i % 128, i // 128, :] = table[flat_idx[i], :]
      flat_idx[i] = idxs_tile[i % 16, i // 16]  (replicated over 8 groups of
      16 partitions, one copy per Q7 core)
    """
    from concourse import library_config

    assert table_rows <= 32767
    nc = bacc.Bacc("TRN2", debug=False)
    idx_d = nc.declare_dram_parameter(
        "idx16", [128, RPC // 16], mybir.dt.int16, isOutput=False
    )
    oids_d = nc.declare_dram_parameter("oids", [RPC], mybir.dt.int32, isOutput=False)
    emb_d = nc.declare_dram_parameter(
        "emb", [table_rows, D], mybir.dt.float32, isOutput=False
    )
    oute_d = nc.declare_dram_parameter(
        "out_emb", [RPC, D], mybir.dt.float32, isOutput=True
    )
    outi_d = nc.declare_dram_parameter("out_ids", [RPC], mybir.dt.int32, isOutput=True)

    K = 2                      # gather/store pipeline chunks
    RK = RPC // K              # rows per chunk
    JK = JPC // K              # 128-row blocks per chunk

    with (
        nc.sbuf_tensor([128, RPC // 16], mybir.dt.int16) as idx_t,
        nc.sbuf_tensor([128, JPC * D], mybir.dt.float32) as g_t,
    ):
        s_idx = nc.alloc_semaphore("s_idx")
        s_g = [nc.alloc_semaphore(f"s_g{k}") for k in range(K)]
        s_o = nc.alloc_semaphore("s_o")
        s_i = nc.alloc_semaphore("s_i")

        nc.scalar.dma_start(idx_t[:, :], idx_d[:, :]).then_inc(s_idx, 16)
        nc.scalar.dma_start(outi_d[:], oids_d[:]).then_inc(s_i, 16)
        nc.scalar.wait_ge(s_i, 16)

        nc.gpsimd.load_library(library_config.mlp)
        nc.gpsimd.wait_ge(s_idx, 16)
        gview = g_t[:, :].rearrange("p (j d) -> p j d", d=D)
        for k in range(K):
            nc.gpsimd.dma_gather(
                gview[:, k * JK : (k + 1) * JK, :],
                emb_d[:, :],
                idx_t[:, k * (RK // 16) : (k + 1) * (RK // 16)],
                RK,
                RK,
                D,
            ).then_inc(s_g[k], 16)

        for k in range(K):
            nc.sync.wait_ge(s_g[k], 16)
            # SBUF partition p holds rows {j*128+p}; scatter them back to
            # row-contiguous DRAM with a strided AP (2 KiB per descriptor)
            dst = oute_d[k * RK : (k + 1) * RK, :].rearrange("(j p) d -> p j d", p=128)
            nc.sync.dma_start(dst, gview[:, k * JK : (k + 1) * JK, :]).then_inc(
                s_o, 16
            )
        nc.sync.wait_ge(s_o, 16 * K)

    nc.compile()
    return nc


def _build_program_indirect(table_rows):
    nc = bass.Bass("TRN2", debug=False)
    idx_d = nc.declare_dram_parameter("idx", [128, JPC], mybir.dt.int32, isOutput=False)
    oids_d = nc.declare_dram_parameter("oids", [RPC], mybir.dt.int32, isOutput=False)
    emb_d = nc.declare_dram_parameter(
        "emb", [table_rows, D], mybir.dt.float32, isOutput=False
    )
    oute_d = nc.declare_dram_parameter(
        "out_emb", [RPC, D], mybir.dt.float32, isOutput=True
    )
    outi_d = nc.declare_dram_parameter("out_ids", [RPC], mybir.dt.int32, isOutput=True)

    # No Block() wrapper: emit every engine's instructions straight into the
    # current basic block.  There is no control flow, inter-engine ordering is
    # fully expressed through semaphores, and skipping the Block-exit
    # all-engine barrier lets the idle engines reach the compiler-injected
    # epilogue (full semaphore sweep) without waiting on an extra barrier.
    with (
        nc.sbuf_tensor([128, JPC], mybir.dt.int32) as idx_t,
        nc.sbuf_tensor([128, JPC * D], mybir.dt.float32) as g_t,
    ):
        s_idx = nc.alloc_semaphore("s_idx")
        s_g = [nc.alloc_semaphore(f"s_g{j}") for j in range(JPC)]
        s_o = nc.alloc_semaphore("s_o")
        s_i = nc.alloc_semaphore("s_i")

        # idx load + id-path copy on the scalar engine's HWDGE queue: issued at
        # kernel start, parallel to everything else
        nc.scalar.dma_start(idx_t[:, :], idx_d[:, :]).then_inc(s_idx, 16)
        nc.scalar.dma_start(outi_d[:], oids_d[:]).then_inc(s_i, 16)
        nc.scalar.wait_ge(s_i, 16)

        # gathers: SWDGE indirect, one row per partition per instruction
        nc.gpsimd.wait_ge(s_idx, 16)
        for j in range(JPC):
            nc.gpsimd.indirect_dma_start(
                out=g_t[:, j * D : (j + 1) * D],
                out_offset=None,
                in_=emb_d[:, :],
                in_offset=bass.IndirectOffsetOnAxis(ap=idx_t[:, j : j + 1], axis=0),
            ).then_inc(s_g[j], 16)

        # stores chase the gathers on the sync engine's HWDGE queue
        for j in range(JPC):
            nc.sync.wait_ge(s_g[j], 16)
            nc.sync.dma_start(
                oute_d[j * 128 : (j + 1) * 128, :], g_t[:, j * D : (j + 1) * D]
            ).then_inc(s_o, 16)
        nc.sync.wait_ge(s_o, 16 * JPC)

    return nc


def _host_index_maps(input_ids, suffix_mask, param, fe_start, fe_len, adv_len,
                     max_left_shift, max_right_shift):
    """O(S) index bookkeeping mirroring the reference's shift semantics."""
    ml, mr = int(max_left_shift), int(max_right_shift)
    F0, F, L = int(fe_start), int(fe_len), int(adv_len)
    Kp = 2 * max(ml, mr) + 1
    p = Kp // 2
    left_pad = max(0, mr - ml)
    right_pad = max(0, ml - mr)
    pk = np.flip(np.pad(param, ((0, 0), (left_pad, right_pad)))[0])

    nz = np.nonzero(pk)[0]
    if len(nz) != 1:
        raise NotImplementedError(
            f"param must be a one-hot shift selector, got {len(nz)} nonzeros"
        )
    k0 = int(nz[0])
    w = float(pk[k0])
    d = k0 - p  # new_fe[t] = w * fe[t + d], zero outside [0, F)

    # ---- embeds path: per-position source index map ----
    s_all = np.arange(S)
    t = s_all - F0
    in_span = (t >= 0) & (t < F)
    valid = in_span & (t + d >= 0) & (t + d < F)
    zero_rows = in_span & ~valid
    src_s = np.where(valid, s_all + d, s_all)

    # gather row index into the (possibly augmented) table
    g = np.take_along_axis(input_ids, np.broadcast_to(src_s, (B, S)), axis=1)
    g = g.astype(np.int32).copy()

    need_zero_row = bool(zero_rows.any())
    need_scale = (w != 1.0)
    table_rows = V
    if need_scale:
        # row i of the scaled copy lives at V + i (+1 past zero row slot)
        g[:, in_span & valid] += V
        table_rows += V
    if need_zero_row:
        g[:, zero_rows] = table_rows
        table_rows += 1

    # ---- id path (mirrors the reference exactly) ----
    ms = p - int(np.argmax(pk == 1.0))
    a0 = np.argmax(np.asarray(suffix_mask), axis=-1).astype(np.int64)
    ns = a0 + ms
    j = np.arange(S)
    oi = np.empty((B, S), dtype=np.int64)
    for b in range(B):
        in_adv = (j >= ns[b]) & (j < ns[b] + L)
        i_non = np.clip(np.where(j < ns[b], j, j - L), 0, S - L - 1)
        src_non = i_non + L * (i_non >= a0[b])
        src_adv = a0[b] + np.clip(j - ns[b], 0, L - 1)
        oi[b] = np.where(in_adv, src_adv, src_non)
    out_ids_vals = np.take_along_axis(input_ids, oi, axis=1).astype(np.int32)

    return g, out_ids_vals, need_zero_row, need_scale, w, table_rows


def kernel(input_ids, suffix_mask, param, emb_weight,
           fe_start, fe_len, adv_len, max_left_shift, max_right_shift):
    input_ids = np.ascontiguousarray(np.asarray(input_ids, dtype=np.int32))
    suffix_mask = np.asarray(suffix_mask)
    param = np.asarray(param, dtype=np.float32)
    emb_weight = np.ascontiguousarray(np.asarray(emb_weight, dtype=np.float32))
    assert input_ids.shape == (B, S) and emb_weight.shape == (V, D)

    g, out_ids_vals, need_zero_row, need_scale, w, table_rows = _host_index_maps(
        input_ids, suffix_mask, param, fe_start, fe_len, adv_len,
        max_left_shift, max_right_shift,
    )

    table = emb_weight
    if need_scale:
        table = np.concatenate([table, emb_weight * np.float32(w)], axis=0)
    if need_zero_row:
        table = np.concatenate([table, np.zeros((1, D), np.float32)], axis=0)
    assert table.shape[0] == table_rows

    fast = table_rows <= 32767
    key = (table_rows, fast)
    if key not in _prog_cache:
        _prog_cache[key] = (
            _build_program(table_rows) if fast else _build_program_indirect(table_rows)
        )
    nc = _prog_cache[key]

    oid_shards = out_ids_vals.reshape(N_CORES, RPC)
    g_flat = g.reshape(N_CORES, RPC)  # local row r = j*128 + p = flat order
    if fast:
        # dma_gather idx tile: [p, s] = flat[s*16 + p%16], replicated over the
        # 8 groups of 16 partitions (one copy per Q7 core)
        idx_shards = [
            np.ascontiguousarray(
                np.tile(g_flat[c].astype(np.int16).reshape(RPC // 16, 16).T, (8, 1))
            )
            for c in range(N_CORES)
        ]
        idx_key = "idx16"
    else:
        # indirect-DMA idx tile: idx_t[p, j] = flat row j*128 + p
        idx_shards = [
            np.ascontiguousarray(g_flat[c].reshape(JPC, 128).T)
            for c in range(N_CORES)
        ]
        idx_key = "idx"
    in_maps = [
        {
            idx_key: idx_shards[c],
            "oids": np.ascontiguousarray(oid_shards[c]),
            "emb": table,
        }
        for c in range(N_CORES)
    ]

    res = run_bass_kernel_spmd(
        nc,
        in_maps,
        core_ids=list(range(N_CORES)),
        trace=TRACE["enabled"],
        **TRACE["kwargs"],
    )
    LAST_RESULTS["res"] = res

    out_embeds = np.concatenate(
        [res.results[c]["out_emb"] for c in range(N_CORES)], axis=0
    ).reshape(B, S, D)
    out_ids = np.concatenate(
        [res.results[c]["out_ids"] for c in range(N_CORES)], axis=0
    ).reshape(B, S)
    return out_embeds.astype(np.float32), out_ids.astype(np.int32)


# revision 13
# speedup vs baseline: 1.2149x; 1.2149x over previous
"""AdversarialBlockShift on 8 TRN2 NeuronCores.

The module's learnable `param` is a one-hot shift selector (a delta at
index `max_left_shift` at init).  After F.pad + flip, the depthwise
conv kernel `pk` is a delta at position k0, so the conv over the user
span is a pure shift by d = k0 - Kp//2 (zeros shifted in at the edge),
and the id path is the matching block move of the adversarial ids.

Everything therefore reduces to one (B*S, D) gather out of the
embedding table with host-computable row indices:

  host:   O(S) int index bookkeeping (shift map, run structure, ids)
  device: the 32 MiB of real memory traffic — gather 8192 rows x 2 KiB
          from the (32000, 512) fp32 table and stream them back out —
          data-parallel over 8 cores, 1024 rows (2+2 MiB) per core.

Per core: SWDGE indirect-DMA gather (128 rows per descriptor batch,
one 2 KiB descriptor per row) into SBUF, HWDGE store to the output
shard, pipelined in chunks so gather-in and store-out overlap.
"""

import numpy as np

import concourse.bass as bass
import concourse.bacc as bacc
from concourse import mybir
from concourse.bass_utils import run_bass_kernel_spmd

# -------- problem constants (hardcoded per contest contract) --------
B, S, D, V = 2, 4096, 512, 32000
N_CORES = 8
ROWS = B * S                      # 8192 flattened output rows
RPC = ROWS // N_CORES             # 1024 rows per core
JPC = RPC // 128                  # 8 row-blocks of 128 per core
# HW indirect DMA gathers ONE row per partition per instruction, so a
# core's 1024 rows take JPC=8 gather instructions of [128, D] each.
# Row mapping (j-major): local row r = j*128 + p  <=>  idx_t[p, j].

# set by test.py for profiling; harness never sets it
TRACE = {"enabled": False, "kwargs": {}}
LAST_RESULTS = {}

_prog_cache = {}


def _build_program(table_rows):
    nc = bass.Bass("TRN2", debug=False, dynamic_dma_scratch_size=140 * 1024)
    idx_d = nc.declare_dram_parameter("idx", [128, JPC], mybir.dt.int32, isOutput=False)
    oids_d = nc.declare_dram_parameter("oids", [RPC], mybir.dt.int32, isOutput=False)
    emb_d = nc.declare_dram_parameter(
        "emb", [table_rows, D], mybir.dt.float32, isOutput=False
    )
    oute_d = nc.declare_dram_parameter(
        "out_emb", [RPC, D], mybir.dt.float32, isOutput=True
    )
    outi_d = nc.declare_dram_parameter("out_ids", [RPC], mybir.dt.int32, isOutput=True)

    # No Block() wrapper: emit every engine's instructions straight into the
    # current basic block.  There is no control flow, inter-engine ordering is
    # fully expressed through semaphores, and skipping the Block-exit
    # all-engine barrier lets the idle engines reach the compiler-injected
    # epilogue (full semaphore sweep) without waiting on an extra barrier.
    with (
        nc.sbuf_tensor([128, JPC], mybir.dt.int32) as idx_t,
        nc.sbuf_tensor([128, JPC * D], mybir.dt.float32) as g_t,
    ):
        s_idx = nc.alloc_semaphore("s_idx")
        s_g = [nc.alloc_semaphore(f"s_g{j}") for j in range(JPC)]
        s_o = nc.alloc_semaphore("s_o")
        s_i = nc.alloc_semaphore("s_i")

        # idx load + id-path copy on the scalar engine's HWDGE queue: issued at
        # kernel start, parallel to everything else
        nc.scalar.dma_start(idx_t[:, :], idx_d[:, :]).then_inc(s_idx, 16)
        nc.scalar.dma_start(outi_d[:], oids_d[:]).then_inc(s_i, 16)
        nc.scalar.wait_ge(s_i, 16)

        # gathers: SWDGE indirect, one row per partition per instruction
        nc.gpsimd.wait_ge(s_idx, 16)
        for j in range(JPC):
            nc.gpsimd.indirect_dma_start(
                out=g_t[:, j * D : (j + 1) * D],
                out_offset=None,
                in_=emb_d[:, :],
                in_offset=bass.IndirectOffsetOnAxis(ap=idx_t[:, j : j + 1], axis=0),
            ).then_inc(s_g[j], 16)

        # stores chase the gathers on the sync engine's HWDGE queue
        for j in range(JPC):
            nc.sync.wait_ge(s_g[j], 16)
            nc.sync.dma_start(
                oute_d[j * 128 : (j + 1) * 128, :], g_t[:, j * D : (j + 1) * D]
            ).then_inc(s_o, 16)
        nc.sync.wait_ge(s_o, 16 * JPC)

    return nc


def _host_index_maps(input_ids, suffix_mask, param, fe_start, fe_len, adv_len,
                     max_left_shift, max_right_shift):
    """O(S) index bookkeeping mirroring the reference's shift semantics."""
    ml, mr = int(max_left_shift), int(max_right_shift)
    F0, F, L = int(fe_start), int(fe_len), int(adv_len)
    Kp = 2 * max(ml, mr) + 1
    p = Kp // 2
    left_pad = max(0, mr - ml)
    right_pad = max(0, ml - mr)
    pk = np.flip(np.pad(param, ((0, 0), (left_pad, right_pad)))[0])

    nz = np.nonzero(pk)[0]
    if len(nz) != 1:
        raise NotImplementedError(
            f"param must be a one-hot shift selector, got {len(nz)} nonzeros"
        )
    k0 = int(nz[0])
    w = float(pk[k0])
    d = k0 - p  # new_fe[t] = w * fe[t + d], zero outside [0, F)

    # ---- embeds path: per-position source index map ----
    s_all = np.arange(S)
    t = s_all - F0
    in_span = (t >= 0) & (t < F)
    valid = in_span & (t + d >= 0) & (t + d < F)
    zero_rows = in_span & ~valid
    src_s = np.where(valid, s_all + d, s_all)

    # gather row index into the (possibly augmented) table
    g = np.take_along_axis(input_ids, np.broadcast_to(src_s, (B, S)), axis=1)
    g = g.astype(np.int32).copy()

    need_zero_row = bool(zero_rows.any())
    need_scale = (w != 1.0)
    table_rows = V
    if need_scale:
        # row i of the scaled copy lives at V + i (+1 past zero row slot)
        g[:, in_span & valid] += V
        table_rows += V
    if need_zero_row:
        g[:, zero_rows] = table_rows
        table_rows += 1

    # ---- id path (mirrors the reference exactly) ----
    ms = p - int(np.argmax(pk == 1.0))
    a0 = np.argmax(np.asarray(suffix_mask), axis=-1).astype(np.int64)
    ns = a0 + ms
    j = np.arange(S)
    oi = np.empty((B, S), dtype=np.int64)
    for b in range(B):
        in_adv = (j >= ns[b]) & (j < ns[b] + L)
        i_non = np.clip(np.where(j < ns[b], j, j - L), 0, S - L - 1)
        src_non = i_non + L * (i_non >= a0[b])
        src_adv = a0[b] + np.clip(j - ns[b], 0, L - 1)
        oi[b] = np.where(in_adv, src_adv, src_non)
    out_ids_vals = np.take_along_axis(input_ids, oi, axis=1).astype(np.int32)

    return g, out_ids_vals, need_zero_row, need_scale, w, table_rows


def kernel(input_ids, suffix_mask, param, emb_weight,
           fe_start, fe_len, adv_len, max_left_shift, max_right_shift):
    input_ids = np.ascontiguousarray(np.asarray(input_ids, dtype=np.int32))
    suffix_mask = np.asarray(suffix_mask)
    param = np.asarray(param, dtype=np.float32)
    emb_weight = np.ascontiguousarray(np.asarray(emb_weight, dtype=np.float32))
    assert input_ids.shape == (B, S) and emb_weight.shape == (V, D)

    g, out_ids_vals, need_zero_row, need_scale, w, table_rows = _host_index_maps(
        input_ids, suffix_mask, param, fe_start, fe_len, adv_len,
        max_left_shift, max_right_shift,
    )

    table = emb_weight
    if need_scale:
        table = np.concatenate([table, emb_weight * np.float32(w)], axis=0)
    if need_zero_row:
        table = np.concatenate([table, np.zeros((1, D), np.float32)], axis=0)
    assert table.shape[0] == table_rows

    if table_rows not in _prog_cache:
        _prog_cache[table_rows] = _build_program(table_rows)
    nc = _prog_cache[table_rows]

    oid_shards = out_ids_vals.reshape(N_CORES, RPC)
    g_flat = g.reshape(N_CORES, RPC)  # local row r = j*128 + p = flat order
    # indirect-DMA idx tile: idx_t[p, j] = flat row j*128 + p
    idx_shards = [
        np.ascontiguousarray(g_flat[c].reshape(JPC, 128).T) for c in range(N_CORES)
    ]
    in_maps = [
        {
            "idx": idx_shards[c],
            "oids": np.ascontiguousarray(oid_shards[c]),
            "emb": table,
        }
        for c in range(N_CORES)
    ]

    res = run_bass_kernel_spmd(
        nc,
        in_maps,
        core_ids=list(range(N_CORES)),
        trace=TRACE["enabled"],
        **TRACE["kwargs"],
    )
    LAST_RESULTS["res"] = res

    out_embeds = np.concatenate(
        [res.results[c]["out_emb"] for c in range(N_CORES)], axis=0
    ).reshape(B, S, D)
    out_ids = np.concatenate(
        [res.results[c]["out_ids"] for c in range(N_CORES)], axis=0
    ).reshape(B, S)
    return out_embeds.astype(np.float32), out_ids.astype(np.int32)


# revision 19
# speedup vs baseline: 1.2809x; 1.0543x over previous
"""AdversarialBlockShift on 8 TRN2 NeuronCores.

The module's learnable `param` is a one-hot shift selector (a delta at
index `max_left_shift` at init).  After F.pad + flip, the depthwise
conv kernel `pk` is a delta at position k0, so the conv over the user
span is a pure shift by d = k0 - Kp//2 (zeros shifted in at the edge),
and the id path is the matching block move of the adversarial ids.

Everything therefore reduces to one (B*S, D) gather out of the
embedding table with host-computable row indices:

  host:   O(S) int index bookkeeping (shift map, run structure, ids)
  device: the 32 MiB of real memory traffic — gather 8192 rows x 2 KiB
          from the (32000, 512) fp32 table and stream them back out —
          data-parallel over 8 cores, 1024 rows (2+2 MiB) per core.

Per core: SWDGE indirect-DMA gather (128 rows per descriptor batch,
one 2 KiB descriptor per row) into SBUF, HWDGE store to the output
shard, pipelined in chunks so gather-in and store-out overlap.
"""

import numpy as np

import concourse.bass as bass
import concourse.bacc as bacc
from concourse import mybir
from concourse.bass_utils import run_bass_kernel_spmd

# -------- problem constants (hardcoded per contest contract) --------
B, S, D, V = 2, 4096, 512, 32000
N_CORES = 8
ROWS = B * S                      # 8192 flattened output rows
RPC = ROWS // N_CORES             # 1024 rows per core
JPC = RPC // 128                  # 8 row-blocks of 128 per core
# HW indirect DMA gathers ONE row per partition per instruction, so a
# core's 1024 rows take JPC=8 gather instructions of [128, D] each.
# Row mapping (j-major): local row r = j*128 + p  <=>  idx_t[p, j].

# set by test.py for profiling; harness never sets it
TRACE = {"enabled": False, "kwargs": {}}
LAST_RESULTS = {}
D2D = {"on": False}   # experimental: DRAM->DRAM indirect gather
N_QUEUES = {"n": 2}   # SWDGE queues to spread gathers over (1..4)

_prog_cache = {}


def _indirect_gather_to_dram(eng, out, in_, in_offset, queue="qPoolDynamic"):
    """BassEngine.indirect_dma_start with two relaxations: the destination may
    be DRAM (unused in the final kernel — faults on HW) and the SWDGE queue is
    selectable (qPoolDynamic{i}) so gathers can spread over multiple queues."""
    offset_ap = in_offset.ap
    offset_axis = in_offset.axis
    assert isinstance(in_.offset, int) and in_.offset == 0
    out_ap = eng.lower_ap_dma(out, for_indirect_dma=True)
    in_ap = eng.lower_ap_dma(in_, for_indirect_dma=True)
    assert len(in_ap) == 1 and len(out_ap) == 1
    offset_lowered = eng.lower_ap_dma(offset_ap)
    assert len(offset_lowered) == 1
    in_ap.append(offset_lowered[0])
    coef = 1
    for i in range(offset_axis + 1, len(in_.shape)):
        coef *= in_.shape[i]
    in_ap[0].dynamic_ap_info = mybir.DynamicAccessPatternInfo(
        c=0,
        actual_ap=out.ap,
        indirect_dim_max_index=in_.shape[offset_axis],
        offset_expr=[
            mybir.DynamicAccessPatternOffsetExpr(
                coef=coef,
                aff_expr=mybir.DynamicAccessPatternOffsetExprAffExpr(
                    kind="IndirectArgId", arg_id=1
                ),
            )
        ],
    )
    return eng.add_instruction(
        mybir.InstDMACopy(
            name=eng.bass.get_next_instruction_name(),
            queue=queue,
            mode="Copy",
            ins=in_ap,
            outs=out_ap,
            oob_is_err=True,
            cce_op=mybir.AluOpType.bypass,
        )
    )


def _build_program(table_rows, d2d=False, n_queues=1):
    nc = bass.Bass(
        "TRN2",
        debug=False,
        dynamic_dma_scratch_size=140 * 1024,
        num_swdge_queues=max(n_queues, 1),
    )
    idx_d = nc.declare_dram_parameter("idx", [128, JPC], mybir.dt.int32, isOutput=False)
    oids_d = nc.declare_dram_parameter("oids", [RPC], mybir.dt.int32, isOutput=False)
    emb_d = nc.declare_dram_parameter(
        "emb", [table_rows, D], mybir.dt.float32, isOutput=False
    )
    oute_d = nc.declare_dram_parameter(
        "out_emb", [RPC, D], mybir.dt.float32, isOutput=True
    )
    outi_d = nc.declare_dram_parameter("out_ids", [RPC], mybir.dt.int32, isOutput=True)

    # No Block() wrapper: emit every engine's instructions straight into the
    # current basic block.  There is no control flow, inter-engine ordering is
    # fully expressed through semaphores, and skipping the Block-exit
    # all-engine barrier lets the idle engines reach the compiler-injected
    # epilogue (full semaphore sweep) without waiting on an extra barrier.
    with (
        nc.sbuf_tensor([128, JPC], mybir.dt.int32) as idx_t,
        nc.sbuf_tensor([128, JPC * D], mybir.dt.float32) as g_t,
    ):
        s_idx = nc.alloc_semaphore("s_idx")
        s_g = [nc.alloc_semaphore(f"s_g{j}") for j in range(JPC)]
        s_o = nc.alloc_semaphore("s_o")
        s_i = nc.alloc_semaphore("s_i")

        # idx load + id-path copy on the scalar engine's HWDGE queue: issued at
        # kernel start, parallel to everything else
        nc.scalar.dma_start(idx_t[:, :], idx_d[:, :]).then_inc(s_idx, 16)
        nc.scalar.dma_start(outi_d[:], oids_d[:]).then_inc(s_i, 16)
        nc.scalar.wait_ge(s_i, 16)

        # gathers: SWDGE indirect, one row per partition per instruction
        nc.gpsimd.wait_ge(s_idx, 16)
        if d2d:
            # gather rows straight into output DRAM: no SBUF bounce, no stores
            for j in range(JPC):
                _indirect_gather_to_dram(
                    nc.gpsimd,
                    out=oute_d[j * 128 : (j + 1) * 128, :],
                    in_=emb_d[:, :],
                    in_offset=bass.IndirectOffsetOnAxis(
                        ap=idx_t[:, j : j + 1], axis=0
                    ),
                ).then_inc(s_g[0], 16)
            nc.gpsimd.wait_ge(s_g[0], 16 * JPC)
        else:
            for j in range(JPC):
                q = j % n_queues
                _indirect_gather_to_dram(
                    nc.gpsimd,
                    out=g_t[:, j * D : (j + 1) * D],
                    in_=emb_d[:, :],
                    in_offset=bass.IndirectOffsetOnAxis(
                        ap=idx_t[:, j : j + 1], axis=0
                    ),
                    queue=f"qPoolDynamic{q or ''}",
                ).then_inc(s_g[j], 16)

            # stores chase the gathers on the sync engine's HWDGE queue
            for j in range(JPC):
                nc.sync.wait_ge(s_g[j], 16)
                nc.sync.dma_start(
                    oute_d[j * 128 : (j + 1) * 128, :], g_t[:, j * D : (j + 1) * D]
                ).then_inc(s_o, 16)
            nc.sync.wait_ge(s_o, 16 * JPC)

    return nc


def _host_index_maps(input_ids, suffix_mask, param, fe_start, fe_len, adv_len,
                     max_left_shift, max_right_shift):
    """O(S) index bookkeeping mirroring the reference's shift semantics."""
    ml, mr = int(max_left_shift), int(max_right_shift)
    F0, F, L = int(fe_start), int(fe_len), int(adv_len)
    Kp = 2 * max(ml, mr) + 1
    p = Kp // 2
    left_pad = max(0, mr - ml)
    right_pad = max(0, ml - mr)
    pk = np.flip(np.pad(param, ((0, 0), (left_pad, right_pad)))[0])

    nz = np.nonzero(pk)[0]
    if len(nz) != 1:
        raise NotImplementedError(
            f"param must be a one-hot shift selector, got {len(nz)} nonzeros"
        )
    k0 = int(nz[0])
    w = float(pk[k0])
    d = k0 - p  # new_fe[t] = w * fe[t + d], zero outside [0, F)

    # ---- embeds path: per-position source index map ----
    s_all = np.arange(S)
    t = s_all - F0
    in_span = (t >= 0) & (t < F)
    valid = in_span & (t + d >= 0) & (t + d < F)
    zero_rows = in_span & ~valid
    src_s = np.where(valid, s_all + d, s_all)

    # gather row index into the (possibly augmented) table
    g = np.take_along_axis(input_ids, np.broadcast_to(src_s, (B, S)), axis=1)
    g = g.astype(np.int32).copy()

    need_zero_row = bool(zero_rows.any())
    need_scale = (w != 1.0)
    table_rows = V
    if need_scale:
        # row i of the scaled copy lives at V + i (+1 past zero row slot)
        g[:, in_span & valid] += V
        table_rows += V
    if need_zero_row:
        g[:, zero_rows] = table_rows
        table_rows += 1

    # ---- id path (mirrors the reference exactly) ----
    ms = p - int(np.argmax(pk == 1.0))
    a0 = np.argmax(np.asarray(suffix_mask), axis=-1).astype(np.int64)
    ns = a0 + ms
    j = np.arange(S)
    oi = np.empty((B, S), dtype=np.int64)
    for b in range(B):
        in_adv = (j >= ns[b]) & (j < ns[b] + L)
        i_non = np.clip(np.where(j < ns[b], j, j - L), 0, S - L - 1)
        src_non = i_non + L * (i_non >= a0[b])
        src_adv = a0[b] + np.clip(j - ns[b], 0, L - 1)
        oi[b] = np.where(in_adv, src_adv, src_non)
    out_ids_vals = np.take_along_axis(input_ids, oi, axis=1).astype(np.int32)

    return g, out_ids_vals, need_zero_row, need_scale, w, table_rows


def kernel(input_ids, suffix_mask, param, emb_weight,
           fe_start, fe_len, adv_len, max_left_shift, max_right_shift):
    input_ids = np.ascontiguousarray(np.asarray(input_ids, dtype=np.int32))
    suffix_mask = np.asarray(suffix_mask)
    param = np.asarray(param, dtype=np.float32)
    emb_weight = np.ascontiguousarray(np.asarray(emb_weight, dtype=np.float32))
    assert input_ids.shape == (B, S) and emb_weight.shape == (V, D)

    g, out_ids_vals, need_zero_row, need_scale, w, table_rows = _host_index_maps(
        input_ids, suffix_mask, param, fe_start, fe_len, adv_len,
        max_left_shift, max_right_shift,
    )

    table = emb_weight
    if need_scale:
        table = np.concatenate([table, emb_weight * np.float32(w)], axis=0)
    if need_zero_row:
        table = np.concatenate([table, np.zeros((1, D), np.float32)], axis=0)
    assert table.shape[0] == table_rows

    key = (table_rows, D2D["on"], N_QUEUES["n"])
    if key not in _prog_cache:
        _prog_cache[key] = _build_program(
            table_rows, d2d=D2D["on"], n_queues=N_QUEUES["n"]
        )
    nc = _prog_cache[key]

    oid_shards = out_ids_vals.reshape(N_CORES, RPC)
    g_flat = g.reshape(N_CORES, RPC)  # local row r = j*128 + p = flat order
    # indirect-DMA idx tile: idx_t[p, j] = flat row j*128 + p
    idx_shards = [
        np.ascontiguousarray(g_flat[c].reshape(JPC, 128).T) for c in range(N_CORES)
    ]
    in_maps = [
        {
            "idx": idx_shards[c],
            "oids": np.ascontiguousarray(oid_shards[c]),
            "emb": table,
        }
        for c in range(N_CORES)
    ]

    res = run_bass_kernel_spmd(
        nc,
        in_maps,
        core_ids=list(range(N_CORES)),
        trace=TRACE["enabled"],
        **TRACE["kwargs"],
    )
    LAST_RESULTS["res"] = res

    out_embeds = np.concatenate(
        [res.results[c]["out_emb"] for c in range(N_CORES)], axis=0
    ).reshape(B, S, D)
    out_ids = np.concatenate(
        [res.results[c]["out_ids"] for c in range(N_CORES)], axis=0
    ).reshape(B, S)
    return out_embeds.astype(np.float32), out_ids.astype(np.int32)


# revision 20
# speedup vs baseline: 1.3316x; 1.0396x over previous
"""AdversarialBlockShift on 8 TRN2 NeuronCores.

The module's learnable `param` is a one-hot shift selector (a delta at
index `max_left_shift` at init).  After F.pad + flip, the depthwise
conv kernel `pk` is a delta at position k0, so the conv over the user
span is a pure shift by d = k0 - Kp//2 (zeros shifted in at the edge),
and the id path is the matching block move of the adversarial ids.

Everything therefore reduces to one (B*S, D) gather out of the
embedding table with host-computable row indices:

  host:   O(S) int index bookkeeping (shift map, run structure, ids)
  device: the 32 MiB of real memory traffic — gather 8192 rows x 2 KiB
          from the (32000, 512) fp32 table and stream them back out —
          data-parallel over 8 cores, 1024 rows (2+2 MiB) per core.

Per core: SWDGE indirect-DMA gather (128 rows per descriptor batch,
one 2 KiB descriptor per row) into SBUF, HWDGE store to the output
shard, pipelined in chunks so gather-in and store-out overlap.
"""

import numpy as np

import concourse.bass as bass
import concourse.bacc as bacc
from concourse import mybir
from concourse.bass_utils import run_bass_kernel_spmd

# -------- problem constants (hardcoded per contest contract) --------
B, S, D, V = 2, 4096, 512, 32000
N_CORES = 8
ROWS = B * S                      # 8192 flattened output rows
RPC = ROWS // N_CORES             # 1024 rows per core
JPC = RPC // 128                  # 8 row-blocks of 128 per core
# HW indirect DMA gathers ONE row per partition per instruction, so a
# core's 1024 rows take JPC=8 gather instructions of [128, D] each.
# Row mapping (j-major): local row r = j*128 + p  <=>  idx_t[p, j].

# set by test.py for profiling; harness never sets it
TRACE = {"enabled": False, "kwargs": {}}
LAST_RESULTS = {}
D2D = {"on": False}   # experimental: DRAM->DRAM indirect gather
N_QUEUES = {"n": 4}   # SWDGE queues to spread gathers over (1..4)

_prog_cache = {}


def _indirect_gather_to_dram(eng, out, in_, in_offset, queue="qPoolDynamic"):
    """BassEngine.indirect_dma_start with two relaxations: the destination may
    be DRAM (unused in the final kernel — faults on HW) and the SWDGE queue is
    selectable (qPoolDynamic{i}) so gathers can spread over multiple queues."""
    offset_ap = in_offset.ap
    offset_axis = in_offset.axis
    assert isinstance(in_.offset, int) and in_.offset == 0
    out_ap = eng.lower_ap_dma(out, for_indirect_dma=True)
    in_ap = eng.lower_ap_dma(in_, for_indirect_dma=True)
    assert len(in_ap) == 1 and len(out_ap) == 1
    offset_lowered = eng.lower_ap_dma(offset_ap)
    assert len(offset_lowered) == 1
    in_ap.append(offset_lowered[0])
    coef = 1
    for i in range(offset_axis + 1, len(in_.shape)):
        coef *= in_.shape[i]
    in_ap[0].dynamic_ap_info = mybir.DynamicAccessPatternInfo(
        c=0,
        actual_ap=out.ap,
        indirect_dim_max_index=in_.shape[offset_axis],
        offset_expr=[
            mybir.DynamicAccessPatternOffsetExpr(
                coef=coef,
                aff_expr=mybir.DynamicAccessPatternOffsetExprAffExpr(
                    kind="IndirectArgId", arg_id=1
                ),
            )
        ],
    )
    return eng.add_instruction(
        mybir.InstDMACopy(
            name=eng.bass.get_next_instruction_name(),
            queue=queue,
            mode="Copy",
            ins=in_ap,
            outs=out_ap,
            oob_is_err=True,
            cce_op=mybir.AluOpType.bypass,
        )
    )


def _build_program(table_rows, d2d=False, n_queues=1):
    nc = bass.Bass(
        "TRN2",
        debug=False,
        dynamic_dma_scratch_size=140 * 1024,
        num_swdge_queues=max(n_queues, 1),
    )
    idx_d = nc.declare_dram_parameter("idx", [128, JPC], mybir.dt.int32, isOutput=False)
    oids_d = nc.declare_dram_parameter("oids", [RPC], mybir.dt.int32, isOutput=False)
    emb_d = nc.declare_dram_parameter(
        "emb", [table_rows, D], mybir.dt.float32, isOutput=False
    )
    oute_d = nc.declare_dram_parameter(
        "out_emb", [RPC, D], mybir.dt.float32, isOutput=True
    )
    outi_d = nc.declare_dram_parameter("out_ids", [RPC], mybir.dt.int32, isOutput=True)

    # No Block() wrapper: emit every engine's instructions straight into the
    # current basic block.  There is no control flow, inter-engine ordering is
    # fully expressed through semaphores, and skipping the Block-exit
    # all-engine barrier lets the idle engines reach the compiler-injected
    # epilogue (full semaphore sweep) without waiting on an extra barrier.
    with (
        nc.sbuf_tensor([128, JPC], mybir.dt.int32) as idx_t,
        nc.sbuf_tensor([128, JPC * D], mybir.dt.float32) as g_t,
    ):
        s_idx = nc.alloc_semaphore("s_idx")
        s_g = [nc.alloc_semaphore(f"s_g{j}") for j in range(JPC)]
        s_o = nc.alloc_semaphore("s_o")
        s_i = nc.alloc_semaphore("s_i")

        # idx load + id-path copy on the scalar engine's HWDGE queue: issued at
        # kernel start, parallel to everything else
        nc.scalar.dma_start(idx_t[:, :], idx_d[:, :]).then_inc(s_idx, 16)
        nc.scalar.dma_start(outi_d[:], oids_d[:]).then_inc(s_i, 16)
        nc.scalar.wait_ge(s_i, 16)

        # gathers: SWDGE indirect, one row per partition per instruction
        nc.gpsimd.wait_ge(s_idx, 16)
        if d2d:
            # gather rows straight into output DRAM: no SBUF bounce, no stores
            for j in range(JPC):
                _indirect_gather_to_dram(
                    nc.gpsimd,
                    out=oute_d[j * 128 : (j + 1) * 128, :],
                    in_=emb_d[:, :],
                    in_offset=bass.IndirectOffsetOnAxis(
                        ap=idx_t[:, j : j + 1], axis=0
                    ),
                ).then_inc(s_g[0], 16)
            nc.gpsimd.wait_ge(s_g[0], 16 * JPC)
        else:
            for j in range(JPC):
                q = j % n_queues
                _indirect_gather_to_dram(
                    nc.gpsimd,
                    out=g_t[:, j * D : (j + 1) * D],
                    in_=emb_d[:, :],
                    in_offset=bass.IndirectOffsetOnAxis(
                        ap=idx_t[:, j : j + 1], axis=0
                    ),
                    queue=f"qPoolDynamic{q or ''}",
                ).then_inc(s_g[j], 16)

            # stores chase the gathers on the sync engine's HWDGE queue
            for j in range(JPC):
                nc.sync.wait_ge(s_g[j], 16)
                nc.sync.dma_start(
                    oute_d[j * 128 : (j + 1) * 128, :], g_t[:, j * D : (j + 1) * D]
                ).then_inc(s_o, 16)
            nc.sync.wait_ge(s_o, 16 * JPC)

    return nc


def _host_index_maps(input_ids, suffix_mask, param, fe_start, fe_len, adv_len,
                     max_left_shift, max_right_shift):
    """O(S) index bookkeeping mirroring the reference's shift semantics."""
    ml, mr = int(max_left_shift), int(max_right_shift)
    F0, F, L = int(fe_start), int(fe_len), int(adv_len)
    Kp = 2 * max(ml, mr) + 1
    p = Kp // 2
    left_pad = max(0, mr - ml)
    right_pad = max(0, ml - mr)
    pk = np.flip(np.pad(param, ((0, 0), (left_pad, right_pad)))[0])

    nz = np.nonzero(pk)[0]
    if len(nz) != 1:
        raise NotImplementedError(
            f"param must be a one-hot shift selector, got {len(nz)} nonzeros"
        )
    k0 = int(nz[0])
    w = float(pk[k0])
    d = k0 - p  # new_fe[t] = w * fe[t + d], zero outside [0, F)

    # ---- embeds path: per-position source index map ----
    s_all = np.arange(S)
    t = s_all - F0
    in_span = (t >= 0) & (t < F)
    valid = in_span & (t + d >= 0) & (t + d < F)
    zero_rows = in_span & ~valid
    src_s = np.where(valid, s_all + d, s_all)

    # gather row index into the (possibly augmented) table
    g = np.take_along_axis(input_ids, np.broadcast_to(src_s, (B, S)), axis=1)
    g = g.astype(np.int32).copy()

    need_zero_row = bool(zero_rows.any())
    need_scale = (w != 1.0)
    table_rows = V
    if need_scale:
        # row i of the scaled copy lives at V + i (+1 past zero row slot)
        g[:, in_span & valid] += V
        table_rows += V
    if need_zero_row:
        g[:, zero_rows] = table_rows
        table_rows += 1

    # ---- id path (mirrors the reference exactly) ----
    ms = p - int(np.argmax(pk == 1.0))
    a0 = np.argmax(np.asarray(suffix_mask), axis=-1).astype(np.int64)
    ns = a0 + ms
    j = np.arange(S)
    oi = np.empty((B, S), dtype=np.int64)
    for b in range(B):
        in_adv = (j >= ns[b]) & (j < ns[b] + L)
        i_non = np.clip(np.where(j < ns[b], j, j - L), 0, S - L - 1)
        src_non = i_non + L * (i_non >= a0[b])
        src_adv = a0[b] + np.clip(j - ns[b], 0, L - 1)
        oi[b] = np.where(in_adv, src_adv, src_non)
    out_ids_vals = np.take_along_axis(input_ids, oi, axis=1).astype(np.int32)

    return g, out_ids_vals, need_zero_row, need_scale, w, table_rows


def kernel(input_ids, suffix_mask, param, emb_weight,
           fe_start, fe_len, adv_len, max_left_shift, max_right_shift):
    input_ids = np.ascontiguousarray(np.asarray(input_ids, dtype=np.int32))
    suffix_mask = np.asarray(suffix_mask)
    param = np.asarray(param, dtype=np.float32)
    emb_weight = np.ascontiguousarray(np.asarray(emb_weight, dtype=np.float32))
    assert input_ids.shape == (B, S) and emb_weight.shape == (V, D)

    g, out_ids_vals, need_zero_row, need_scale, w, table_rows = _host_index_maps(
        input_ids, suffix_mask, param, fe_start, fe_len, adv_len,
        max_left_shift, max_right_shift,
    )

    table = emb_weight
    if need_scale:
        table = np.concatenate([table, emb_weight * np.float32(w)], axis=0)
    if need_zero_row:
        table = np.concatenate([table, np.zeros((1, D), np.float32)], axis=0)
    assert table.shape[0] == table_rows

    key = (table_rows, D2D["on"], N_QUEUES["n"])
    if key not in _prog_cache:
        _prog_cache[key] = _build_program(
            table_rows, d2d=D2D["on"], n_queues=N_QUEUES["n"]
        )
    nc = _prog_cache[key]

    oid_shards = out_ids_vals.reshape(N_CORES, RPC)
    g_flat = g.reshape(N_CORES, RPC)  # local row r = j*128 + p = flat order
    # indirect-DMA idx tile: idx_t[p, j] = flat row j*128 + p
    idx_shards = [
        np.ascontiguousarray(g_flat[c].reshape(JPC, 128).T) for c in range(N_CORES)
    ]
    in_maps = [
        {
            "idx": idx_shards[c],
            "oids": np.ascontiguousarray(oid_shards[c]),
            "emb": table,
        }
        for c in range(N_CORES)
    ]

    res = run_bass_kernel_spmd(
        nc,
        in_maps,
        core_ids=list(range(N_CORES)),
        trace=TRACE["enabled"],
        **TRACE["kwargs"],
    )
    LAST_RESULTS["res"] = res

    out_embeds = np.concatenate(
        [res.results[c]["out_emb"] for c in range(N_CORES)], axis=0
    ).reshape(B, S, D)
    out_ids = np.concatenate(
        [res.results[c]["out_ids"] for c in range(N_CORES)], axis=0
    ).reshape(B, S)
    return out_embeds.astype(np.float32), out_ids.astype(np.int32)
